# revision 1
# baseline (speedup 1.0000x reference)
"""ChebyKAN layer kernel for Trainium2 (8 NeuronCores).

Computes y[b,o] = sum_{i,d} T_d(tanh(x)[b,i]) * C[i,o,d] for
x: (8192, 1024) f32, C: (1024, 1024, 9) f32, i.e. a (8192 x 9216) @
(9216 x 1024) matmul after building Chebyshev features.

Sharding: 4-way over batch rows x 2-way over output columns
(core c -> batch group c//2, output group c%2). No collectives.

On-chip math: the Chebyshev basis is replaced by a product-feature basis
that needs only 1 multiply (+ occasional affine) per degree:
    F1 = t            = T1
    F2 = t*t          = (T2+1)/2        t2 = 2*F2-1 = T2
    F3 = t*t2         = (T3+T1)/2
    F4 = t2*t2        = (T4+1)/2        t4 = 2*F4-1 = T4
    F5 = t*t4         = (T5+T3)/2
    F6 = t2*t4        = (T6+T2)/2
    F7 = t4*F3        = (T7+T5+T3+T1)/4
    F8 = t4*t4        = (T8+1)/2
F is a triangular, well-conditioned linear transform of T, so the
weights are re-expressed host-side: y = sum_k F_k V_k with
    V0 = W0 - W2 - W4 + W6 - W8   (constant feature -> folded bias)
    V1 = W1 - W3 + W5 - W7
    V2 = 2(W2 - W6);  V3 = 2(W3 - W5);  V4 = 2 W4
    V5 = 2(W5 - W7);  V6 = 2 W6;  V7 = 4 W7;  V8 = 2 W8
The constant-feature term sum_i V0[i,o] is computed by one ones-row
matmul against V0 pre-reduced to 128 rows host-side (saves 1/9 of the
matmul work). Matmul runs in bf16 (f32 PSUM accumulate); the feature
chain runs in f32 on-chip.
"""

from contextlib import ExitStack

import ml_dtypes
import numpy as np

import concourse.bacc as bacc
import concourse.mybir as mybir
import concourse.tile as tile
from concourse.bass_utils import run_bass_kernel_spmd

P = 128
B_FULL, I_DIM, O_FULL, DEG = 8192, 1024, 1024, 8
N_CORES = 8
BG, OG = 4, 2  # core grid: batch groups x output groups
B_SH = B_FULL // BG  # 2048 batch rows per core
O_SH = O_FULL // OG  # 512 output cols per core
CH = 256  # batch-chunk width per phase
N_PH = B_SH // CH  # 4 phases
NBT = CH // P  # 4 psum b-tiles per phase
NIT = I_DIM // P  # 8 i-tiles
KT = 1 + DEG * NIT  # 65 contraction tiles (1 ones + 64 features)
F32 = mybir.dt.float32
F16 = mybir.dt.float16
BF16 = mybir.dt.bfloat16
MULT = mybir.AluOpType.mult
ADD = mybir.AluOpType.add
TANH = mybir.ActivationFunctionType.Tanh

_NC_CACHE = []


def _build_ir(repeat=1, loop_iters=None, variant="full"):
    nc = bacc.Bacc(
        "TRN2", target_bir_lowering=False, debug=False, enable_asserts=False
    )
    xT = nc.dram_tensor("xT", [I_DIM, B_SH], F32, kind="ExternalInput").ap()
    wv = nc.dram_tensor("wv", [KT * P, O_SH], BF16, kind="ExternalInput").ap()
    y = nc.dram_tensor("y", [B_SH, O_SH], F32, kind="ExternalOutput").ap()

    with ExitStack() as ctx:
        tc = ctx.enter_context(tile.TileContext(nc))
        wpool = ctx.enter_context(tc.tile_pool(name="w", bufs=1))
        fpool = ctx.enter_context(tc.tile_pool(name="f", bufs=2))
        cpool = ctx.enter_context(tc.tile_pool(name="c", bufs=6))
        xpool = ctx.enter_context(tc.tile_pool(name="x", bufs=8))
        ypool = ctx.enter_context(tc.tile_pool(name="yp", bufs=8))
        opool = ctx.enter_context(tc.tile_pool(name="o", bufs=1))
        pspool = ctx.enter_context(tc.tile_pool(name="ps", bufs=8, space="PSUM"))

        ones = opool.tile([P, P], BF16, tag="ones")
        nc.vector.memset(ones[:], 1.0)

        wt = [wpool.tile([P, O_SH], BF16, tag=f"w{k}", name=f"w{k}") for k in range(KT)]
        # Weight DMAs are emitted interleaved with phase 0's x loads (in PE
        # consumption order) so the first phase's features aren't starved
        # behind the full 8.5 MB weight load.
        nc.sync.dma_start(out=wt[0][:], in_=wv[0:P, :])
        if loop_iters is not None:
            # timing variant: weights fully loaded before the hw loop
            for k in range(1, KT):
                nc.sync.dma_start(out=wt[k][:], in_=wv[k * P : (k + 1) * P, :])

        fixed_feat = {}
        if variant == "pe":
            # PE-only: features are static tiles memset once up front
            fpool_pe = ctx.enter_context(tc.tile_pool(name="fpe", bufs=1))
            for it in range(NIT):
                for d in range(1, DEG + 1):
                    t = fpool_pe.tile([P, CH], BF16, tag=f"pf{d}_{it}",
                                      name=f"pf{d}_{it}")
                    nc.vector.memset(t[:], 0.01 * d)
                    fixed_feat[(d, it)] = t

        def emit_body(rep):
          for ph in range(N_PH):
            b0 = ph * CH
            do_mm = variant in ("full", "pe")
            do_prod = variant in ("full", "prod")
            psums = [pspool.tile([P, O_SH], F32, tag="ps", name="ps") for _ in range(NBT)]
            if do_mm:
                for bt in range(NBT):
                    nc.tensor.matmul(
                        psums[bt][:], ones[:], wt[0][:], start=True, stop=False
                    )
            for it in range(NIT):
                f = [None] * (DEG + 1)
                if do_prod:
                    xr = xpool.tile([P, CH], F32, tag="xr")
                    nc.sync.dma_start(out=xr[:], in_=xT[it * P : (it + 1) * P, b0 : b0 + CH])
                    if ph == 0 and rep == 0 and loop_iters is None:
                        for d in range(1, DEG + 1):
                            k = 1 + (d - 1) * NIT + it
                            nc.sync.dma_start(out=wt[k][:], in_=wv[k * P : (k + 1) * P, :])
                    # fp16 chain: Chebyshev features live in [-1, 1], so
                    # fp16's 2^-11 absolute error beats the final bf16
                    # feature rounding. 16-bit dtypes unlock DVE 2x/4x modes.
                    t1 = cpool.tile([P, CH], F16, tag="t1")
                    nc.scalar.activation(t1[:], xr[:], TANH)

                    def featd(d, src):
                        f[d] = fpool.tile([P, CH], BF16, tag=f"f{d}_{it}", name=f"f{d}_{it}")
                        nc.vector.tensor_copy(f[d][:], src[:])

                    def feata(d, src):
                        f[d] = fpool.tile([P, CH], BF16, tag=f"f{d}_{it}", name=f"f{d}_{it}")
                        nc.scalar.copy(f[d][:], src[:])

                    featd(1, t1)
                    sq1 = cpool.tile([P, CH], F16, tag="sq1")
                    nc.vector.tensor_tensor(sq1[:], t1[:], t1[:], MULT)
                    featd(2, sq1)
                    t2 = cpool.tile([P, CH], F16, tag="t2")
                    nc.vector.tensor_scalar(t2[:], sq1[:], 2.0, -1.0, MULT, ADD)
                    p3 = cpool.tile([P, CH], F16, tag="p3")
                    nc.vector.tensor_tensor(p3[:], t1[:], t2[:], MULT)
                    feata(3, p3)
                    sq2 = cpool.tile([P, CH], F16, tag="sq2")
                    nc.vector.tensor_tensor(sq2[:], t2[:], t2[:], MULT)
                    feata(4, sq2)
                    t4 = cpool.tile([P, CH], F16, tag="t4")
                    nc.vector.tensor_scalar(t4[:], sq2[:], 2.0, -1.0, MULT, ADD)
                    for d, (a, b) in ((5, (t1, t4)), (6, (t2, t4)), (7, (t4, p3)), (8, (t4, t4))):
                        f[d] = fpool.tile([P, CH], BF16, tag=f"f{d}_{it}", name=f"f{d}_{it}")
                        nc.vector.tensor_tensor(f[d][:], a[:], b[:], MULT)
                else:
                    for d in range(1, DEG + 1):
                        f[d] = fixed_feat[(d, it)]

                if do_mm:
                    for d in range(1, DEG + 1):
                        k = 1 + (d - 1) * NIT + it
                        last = it == NIT - 1 and d == DEG
                        for bt in range(NBT):
                            nc.tensor.matmul(
                                psums[bt][:],
                                f[d][:, bt * P : (bt + 1) * P],
                                wt[k][:],
                                start=False,
                                stop=last,
                            )
            for bt in range(NBT):
                ysb = ypool.tile([P, O_SH], F32, tag="ysb")
                if do_mm:
                    nc.scalar.copy(ysb[:], psums[bt][:])
                else:
                    nc.scalar.copy(ysb[:], f[8][:])
                nc.sync.dma_start(
                    out=y[b0 + bt * P : b0 + (bt + 1) * P, :], in_=ysb[:]
                )

        if loop_iters is not None:
            with tc.For_i(0, loop_iters, 1):
                emit_body(0)
        else:
            for rep in range(repeat):
                emit_body(rep)
    nc.compile()
    return nc


def get_nc():
    if not _NC_CACHE:
        _NC_CACHE.append(_build_ir())
    return _NC_CACHE[0]


def prep_inputs(x, cheby_coeffs):
    """Host-side shard prep: returns per-core input maps."""
    x = np.asarray(x, dtype=np.float32)
    c = np.asarray(cheby_coeffs, dtype=np.float64)
    w = [c[:, :, d] for d in range(DEG + 1)]
    v = [
        w[0] - w[2] - w[4] + w[6] - w[8],
        w[1] - w[3] + w[5] - w[7],
        2.0 * (w[2] - w[6]),
        2.0 * (w[3] - w[5]),
        2.0 * w[4],
        2.0 * (w[5] - w[7]),
        2.0 * w[6],
        4.0 * w[7],
        2.0 * w[8],
    ]
    v0r = v[0].reshape(NIT, P, O_FULL).sum(axis=0)  # (128, 1024)
    wv_full = np.concatenate([v0r] + v[1:], axis=0)  # (8320, 1024)
    wv_bf = wv_full.astype(ml_dtypes.bfloat16)
    xt_full = np.ascontiguousarray(x.T)  # (1024, 8192)

    in_maps = []
    for core in range(N_CORES):
        bg, og = core // OG, core % OG
        in_maps.append(
            {
                "xT": np.ascontiguousarray(
                    xt_full[:, bg * B_SH : (bg + 1) * B_SH]
                ),
                "wv": np.ascontiguousarray(
                    wv_bf[:, og * O_SH : (og + 1) * O_SH]
                ),
            }
        )
    return in_maps


def assemble_output(results):
    y_full = np.empty((B_FULL, O_FULL), dtype=np.float32)
    for core in range(N_CORES):
        bg, og = core // OG, core % OG
        y_full[bg * B_SH : (bg + 1) * B_SH, og * O_SH : (og + 1) * O_SH] = (
            np.asarray(results[core]["y"], dtype=np.float32)
        )
    return y_full


def kernel(x, cheby_coeffs):
    nc = get_nc()
    in_maps = prep_inputs(x, cheby_coeffs)
    res = run_bass_kernel_spmd(nc, in_maps, list(range(N_CORES)))
    return assemble_output(res.results)



# revision 5
# speedup vs baseline: 1.5695x; 1.5695x over previous
"""ChebyKAN layer kernel for Trainium2 (8 NeuronCores).

Computes y[b,o] = sum_{i,d} T_d(tanh(x)[b,i]) * C[i,o,d] for
x: (8192, 1024) f32, C: (1024, 1024, 9) f32, i.e. a (8192 x 8192) @
(8192 x 1024) matmul after building product features, plus a constant
bias row (the degree-0 term) added during the PSUM drain.

Sharding: 8-way over batch rows, weights replicated. Each core computes
all 1024 outputs for its 1024 rows, so the tanh/feature chain runs once
per batch row. No collectives.

On-chip math: the Chebyshev basis is replaced by a product-feature basis
that needs only 1 multiply (+ occasional affine) per degree:
    F1 = t            = T1
    F2 = t*t          = (T2+1)/2        t2 = 2*F2-1 = T2
    F3 = t*t2         = (T3+T1)/2
    F4 = t2*t2        = (T4+1)/2        t4 = 2*F4-1 = T4
    F5 = t*t4         = (T5+T3)/2
    F6 = t2*t4        = (T6+T2)/2
    F7 = t4*F3        = (T7+T5+T3+T1)/4
    F8 = t4*t4        = (T8+1)/2
F is a triangular, well-conditioned linear transform of T, so the
weights are re-expressed host-side: y = bias + sum_k F_k V_k with
    V0 = W0 - W2 - W4 + W6 - W8   (constant feature -> bias row)
    V1 = W1 - W3 + W5 - W7
    V2 = 2(W2 - W6);  V3 = 2(W3 - W5);  V4 = 2 W4
    V5 = 2(W5 - W7);  V6 = 2 W6;  V7 = 4 W7;  V8 = 2 W8
    bias[o] = sum_i V0[i,o]  (precomputed host-side, replicated to all
    128 partitions, added on the DVE while draining PSUM)
This makes the PE work exactly the 8192-deep contraction: 1024 matmuls
of 512 columns per exec — the MAC-count minimum. Matmul runs in bf16
(f32 PSUM accumulate); the feature chain runs in f32/fp16 on-chip.
Each feature block is the stationary operand for two 512-wide matmuls
(the two output halves).
"""

from contextlib import ExitStack

import ml_dtypes
import numpy as np

import concourse.bacc as bacc
import concourse.mybir as mybir
import concourse.tile as tile
from concourse.bass_utils import run_bass_kernel_spmd

P = 128
B_FULL, I_DIM, O_FULL, DEG = 8192, 1024, 1024, 8
N_CORES = 8
BG = 8  # pure batch sharding
B_SH = B_FULL // BG  # 1024 batch rows per core
O_SH = O_FULL  # full output width per core
NOH = 2  # output halves (512-wide matmuls; psum-bank limit)
OW = O_SH // NOH  # 512
CH = 256  # batch-chunk width per phase
N_PH = B_SH // CH  # 4 phases
NBT = CH // P  # 2 psum b-tiles per phase
NIT = I_DIM // P  # 8 i-tiles
KT = DEG * NIT  # 64 contraction tiles
F32 = mybir.dt.float32
F16 = mybir.dt.float16
BF16 = mybir.dt.bfloat16
MULT = mybir.AluOpType.mult
ADD = mybir.AluOpType.add
TANH = mybir.ActivationFunctionType.Tanh

_NC_CACHE = []


def _build_ir(repeat=1, loop_iters=None, variant="full"):
    nc = bacc.Bacc(
        "TRN2", target_bir_lowering=False, debug=False, enable_asserts=False
    )
    xT = nc.dram_tensor("xT", [I_DIM, B_SH], F32, kind="ExternalInput").ap()
    wv = nc.dram_tensor("wv", [KT * P, O_SH], BF16, kind="ExternalInput").ap()
    bias = nc.dram_tensor("bias", [P, O_SH], F32, kind="ExternalInput").ap()
    y = nc.dram_tensor("y", [B_SH, O_SH], F32, kind="ExternalOutput").ap()

    with ExitStack() as ctx:
        tc = ctx.enter_context(tile.TileContext(nc))
        wpool = ctx.enter_context(tc.tile_pool(name="w", bufs=1))
        fpool = ctx.enter_context(tc.tile_pool(name="f", bufs=1))
        cpool = ctx.enter_context(tc.tile_pool(name="c", bufs=2))
        xpool = ctx.enter_context(tc.tile_pool(name="x", bufs=4))
        ypool = ctx.enter_context(tc.tile_pool(name="yp", bufs=4))
        bpool = ctx.enter_context(tc.tile_pool(name="b", bufs=1))
        pspool = ctx.enter_context(tc.tile_pool(name="ps", bufs=8, space="PSUM"))

        bt_sb = bpool.tile([P, O_SH], F32, tag="bias")
        nc.sync.dma_start(out=bt_sb[:], in_=bias[:, :])

        wt = [wpool.tile([P, O_SH], BF16, tag=f"w{k}", name=f"w{k}") for k in range(KT)]
        # Weight DMAs are emitted interleaved with phase 0's x loads (in PE
        # consumption order) so the first phase's features aren't starved
        # behind the full 17 MB weight load.
        if loop_iters is not None:
            # timing variant: weights fully loaded before the hw loop
            for k in range(KT):
                nc.sync.dma_start(out=wt[k][:], in_=wv[k * P : (k + 1) * P, :])

        fixed_feat = {}
        if variant == "pe":
            # PE-only: features are static tiles memset once up front
            fpool_pe = ctx.enter_context(tc.tile_pool(name="fpe", bufs=1))
            for it in range(NIT):
                for d in range(1, DEG + 1):
                    t = fpool_pe.tile([P, CH], BF16, tag=f"pf{d}_{it}",
                                      name=f"pf{d}_{it}")
                    nc.vector.memset(t[:], 0.01 * d)
                    fixed_feat[(d, it)] = t

        def emit_body(rep):
          for ph in range(N_PH):
            b0 = ph * CH
            do_mm = variant in ("full", "pe")
            do_prod = variant in ("full", "prod")
            psums = [
                pspool.tile([P, OW], F32, tag="ps", name="ps")
                for _ in range(NBT * NOH)
            ]
            for it in range(NIT):
                f = [None] * (DEG + 1)
                if do_prod:
                    xr = xpool.tile([P, CH], F32, tag="xr")
                    nc.sync.dma_start(out=xr[:], in_=xT[it * P : (it + 1) * P, b0 : b0 + CH])
                    if ph == 0 and rep == 0 and loop_iters is None:
                        for d in range(1, DEG + 1):
                            k = (d - 1) * NIT + it
                            nc.sync.dma_start(out=wt[k][:], in_=wv[k * P : (k + 1) * P, :])
                    # fp16 chain: Chebyshev features live in [-1, 1], so
                    # fp16's 2^-11 absolute error beats the final bf16
                    # feature rounding. 16-bit dtypes unlock DVE 2x/4x modes.
                    t1 = cpool.tile([P, CH], F16, tag="t1")
                    nc.scalar.activation(t1[:], xr[:], TANH)

                    def featd(d, src):
                        f[d] = fpool.tile([P, CH], BF16, tag=f"f{d}_{it}", name=f"f{d}_{it}")
                        nc.vector.tensor_copy(f[d][:], src[:])

                    def feata(d, src):
                        f[d] = fpool.tile([P, CH], BF16, tag=f"f{d}_{it}", name=f"f{d}_{it}")
                        nc.scalar.copy(f[d][:], src[:])

                    featd(1, t1)
                    sq1 = cpool.tile([P, CH], F16, tag="sq1")
                    nc.vector.tensor_tensor(sq1[:], t1[:], t1[:], MULT)
                    featd(2, sq1)
                    t2 = cpool.tile([P, CH], F16, tag="t2")
                    nc.vector.tensor_scalar(t2[:], sq1[:], 2.0, -1.0, MULT, ADD)
                    p3 = cpool.tile([P, CH], F16, tag="p3")
                    nc.vector.tensor_tensor(p3[:], t1[:], t2[:], MULT)
                    feata(3, p3)
                    sq2 = cpool.tile([P, CH], F16, tag="sq2")
                    nc.vector.tensor_tensor(sq2[:], t2[:], t2[:], MULT)
                    feata(4, sq2)
                    t4 = cpool.tile([P, CH], F16, tag="t4")
                    nc.vector.tensor_scalar(t4[:], sq2[:], 2.0, -1.0, MULT, ADD)
                    for d, (a, b) in ((5, (t1, t4)), (6, (t2, t4)), (7, (t4, p3)), (8, (t4, t4))):
                        f[d] = fpool.tile([P, CH], BF16, tag=f"f{d}_{it}", name=f"f{d}_{it}")
                        nc.vector.tensor_tensor(f[d][:], a[:], b[:], MULT)
                else:
                    for d in range(1, DEG + 1):
                        f[d] = fixed_feat[(d, it)]

                if do_mm:
                    for d in range(1, DEG + 1):
                        k = (d - 1) * NIT + it
                        first = it == 0 and d == 1
                        last = it == NIT - 1 and d == DEG
                        for bt in range(NBT):
                            for oh in range(NOH):
                                nc.tensor.matmul(
                                    psums[bt * NOH + oh][:],
                                    f[d][:, bt * P : (bt + 1) * P],
                                    wt[k][:, oh * OW : (oh + 1) * OW],
                                    start=first,
                                    stop=last,
                                )
            for bt in range(NBT):
                for oh in range(NOH):
                    ysb = ypool.tile([P, OW], F32, tag="ysb")
                    if do_mm:
                        # drain on Act, then constant-term bias add on DVE
                        # (keeps the PSUM free-up off the DVE queue)
                        nc.scalar.copy(ysb[:], psums[bt * NOH + oh][:])
                        nc.vector.tensor_tensor(
                            ysb[:],
                            ysb[:],
                            bt_sb[:, oh * OW : (oh + 1) * OW],
                            ADD,
                        )
                    else:
                        nc.scalar.copy(ysb[:, :CH], f[8][:])
                    nc.sync.dma_start(
                        out=y[b0 + bt * P : b0 + (bt + 1) * P, oh * OW : (oh + 1) * OW],
                        in_=ysb[:],
                    )

        if loop_iters is not None:
            with tc.For_i(0, loop_iters, 1):
                emit_body(0)
        else:
            for rep in range(repeat):
                emit_body(rep)
    nc.compile()
    return nc


def get_nc():
    if not _NC_CACHE:
        _NC_CACHE.append(_build_ir())
    return _NC_CACHE[0]


def prep_inputs(x, cheby_coeffs):
    """Host-side shard prep: returns per-core input maps."""
    x = np.asarray(x, dtype=np.float32)
    c = np.asarray(cheby_coeffs, dtype=np.float64)
    w = [c[:, :, d] for d in range(DEG + 1)]
    v = [
        w[0] - w[2] - w[4] + w[6] - w[8],
        w[1] - w[3] + w[5] - w[7],
        2.0 * (w[2] - w[6]),
        2.0 * (w[3] - w[5]),
        2.0 * w[4],
        2.0 * (w[5] - w[7]),
        2.0 * w[6],
        4.0 * w[7],
        2.0 * w[8],
    ]
    bias_row = v[0].sum(axis=0)  # (1024,)
    bias_rep = np.ascontiguousarray(
        np.broadcast_to(bias_row[None, :], (P, O_FULL)), dtype=np.float32
    )
    # weight rows ordered degree-major: k = (d-1)*NIT + it
    wv_full = np.concatenate(v[1:], axis=0)  # (8192, 1024)
    wv_bf = np.ascontiguousarray(wv_full.astype(ml_dtypes.bfloat16))
    xt_full = np.ascontiguousarray(x.T)  # (1024, 8192)

    in_maps = []
    for core in range(N_CORES):
        in_maps.append(
            {
                "xT": np.ascontiguousarray(
                    xt_full[:, core * B_SH : (core + 1) * B_SH]
                ),
                "wv": wv_bf,
                "bias": bias_rep,
            }
        )
    return in_maps


def assemble_output(results):
    y_full = np.empty((B_FULL, O_FULL), dtype=np.float32)
    for core in range(N_CORES):
        y_full[core * B_SH : (core + 1) * B_SH, :] = (
            np.asarray(results[core]["y"], dtype=np.float32)
        )
    return y_full


def kernel(x, cheby_coeffs):
    nc = get_nc()
    in_maps = prep_inputs(x, cheby_coeffs)
    res = run_bass_kernel_spmd(nc, in_maps, list(range(N_CORES)))
    return assemble_output(res.results)
